# revision 35
# baseline (speedup 1.0000x reference)
"""Multi-head attention (B=2,S=4096,E=768,H=12,D=64 + 16-token K/V prompt
prefix) on 8 Trainium2 NeuronCores.

Sharding: 2 batches x 4 head-groups (3 heads each). Each core computes QKV
projections for its 3 heads, full attention over its batch, and a partial
output projection (its 192 ctx channels); the host sums the 4 partials per
batch.

Per-core kernel layout (cost model: matmul = out-free-size rows; activation/
DVE/Pool = free-size * cycle):
  qT[c,s]   = Wq_g @ query^T            (lhsT=Wq_g^T chunks, rhs=queryT chunks)
  kT[c,s]   likewise; prompt K prefix kept separate (kpT)
  v[s,c]    natural orientation          (lhsT=valueT chunks, rhs=Wv_g^T)
  scoresT[k,q] = kT-tile^T @ qT          (lhsT=kT tile [64,128], rhs=qT [64,512])
  expT = Exp(scoresT / 8)                (ScalarE, or DVE/Pool via the
                                          (1+u+u^2/2)^8 chain, u=s/64)
  ctx[q, d+denom] += expT-slice^T @ v    (STATIONARY-SWAPPED: lhsT=expT
                                          [128kv,128q], rhs=v_aug [128kv,65]
                                          -> only 65 pumped rows per k-tile
                                          per q-chunk, half the PE cost of the
                                          [d,q] orientation)
  ctx_n[q,c] = ctx * recip(denom[q])     (per-partition scalar multiply)
  ctxT[c,q] = PE-transpose(ctx_n)        (via identity; rides scores psum ring)
  outT[e,q] partial = Wo_g^T-slices @ ctxT

Exp tiles are statically split across ScalarE / DVE / Pool so every engine's
modeled busy time lands near the PE's ~340us. Flexible evacuation ops
(normalize, psum->sbuf copies, out-proj evac) go through a greedy busy-time
balancer across DVE/Pool/ScalarE.
"""

import sys
import threading

import numpy as np

if "/opt/trn_rl_repo" not in sys.path:
    sys.path.insert(0, "/opt/trn_rl_repo")

import ml_dtypes

BF16 = ml_dtypes.bfloat16

B, S, E, H, D, PP = 2, 4096, 768, 12, 64, 16
NCORES = 8
NG = 4          # head-groups (tensor parallel)
HL = H // NG    # 3 local heads
CL = HL * D     # 192 local channels
SKV = PP + S    # 4112
NKT = S // 128  # 32 full k-tiles (prefix handled separately)
QT = 1024       # q tile width for scores/exp
NSQ = S // QT   # 4
NQC = QT // 128  # 8 q-chunks of 128 within a q tile
TRAIL = 16      # ctx matmuls trail scores by this many slots
GAP = 6         # extra trail for block-opening ctx (psc single-buffered)
NST = S // 128  # 32 v stiles

# --- engine split knobs ----------------------------------------------------
# (Pool/GPSIMD cannot access PSUM, so Pool-assigned exp tiles have their
#  first op (psum read) done by DVE; Pool runs the 6 SBUF-only chain ops.)
N_DVE_EXP = 0    # exp tiles handled end-to-end by the DVE 6-op chain
N_POOL_EXP = 0   # exp tiles: DVE op1 + Pool 6-op SBUF tail
# (Measured: every chain-offloaded tile ends up costing ~2.5us of scores-ring
#  stall -- the chain's first op queues behind DVE's in-order backlog and the
#  2-slot scores psum ring serializes on first-reader completion.  ScalarE-only
#  exp is the fastest measured configuration; the ctx stationary-swap keeps PE
#  ~80us below the baseline so ScalarE stays the sole pacer.)

# modeled per-op busy-time (ns) helpers for the greedy balancer
def _cS(f):       # ScalarE activation
    return (f + 222) * 0.8333

def _cD(f):       # DVE tensor op (psum involved)
    return f * 1.0417 + 125.0

def _cP(f, eff=0.6):  # Pool tensor op
    return f * 0.8333 / eff + 95.0

_lock = threading.Lock()
_compiled = {}


def _exp_engine_map():
    """Static slot -> engine ('S'/'D'/'P') assignment for the 384 exp tiles."""
    n_slots = NSQ * HL * NKT
    eng = ["S"] * n_slots
    # Pool tiles: long latency chain (13+ slots) -> keep away from stream ends
    for i in range(N_POOL_EXP):
        s = 40 + int(i * (n_slots - 108) / max(N_POOL_EXP, 1))
        eng[s] = "P"
    for i in range(N_DVE_EXP):
        s = 36 + int(i * (n_slots - 80) / max(N_DVE_EXP, 1))
        while eng[s] != "S":
            s += 1
        eng[s] = "D"
    return eng


def _build():
    import concourse.bass as bass  # noqa: F401
    import concourse.mybir as mybir
    import concourse.tile as tile
    from concourse import bacc

    f32 = mybir.dt.float32
    bf16 = mybir.dt.bfloat16
    EXP = mybir.ActivationFunctionType.Exp
    ALU = mybir.AluOpType

    nc = bacc.Bacc("TRN2", target_bir_lowering=False, debug=False)

    xqT = nc.dram_tensor("xqT", [E, S], bf16, kind="ExternalInput").ap()
    xkT = nc.dram_tensor("xkT", [E, S], bf16, kind="ExternalInput").ap()
    xvT = nc.dram_tensor("xvT", [E, S], bf16, kind="ExternalInput").ap()
    wqT = nc.dram_tensor("wqT", [E, CL], bf16, kind="ExternalInput").ap()
    wkT = nc.dram_tensor("wkT", [E, CL], bf16, kind="ExternalInput").ap()
    wvT = nc.dram_tensor("wvT", [E, CL], bf16, kind="ExternalInput").ap()
    woT = nc.dram_tensor("woT", [CL, E], bf16, kind="ExternalInput").ap()
    bq = nc.dram_tensor("bq", [CL, 1], f32, kind="ExternalInput").ap()
    bk = nc.dram_tensor("bk", [CL, 1], f32, kind="ExternalInput").ap()
    bv = nc.dram_tensor("bv", [1, CL], f32, kind="ExternalInput").ap()
    kpT = nc.dram_tensor("kpT", [128, 2, PP], bf16, kind="ExternalInput").ap()
    # vp3[:, h, :]: head h prefix V at rows 16h..16h+16 (zeros elsewhere),
    # col 64 = ones on those rows.  Zero-padding makes the prefix-ctx matmul
    # a plain full-contraction matmul (other heads' expp rows hit zeros).
    vp3 = nc.dram_tensor("vp3", [96, HL, D + 1], bf16,
                         kind="ExternalInput").ap()
    ident = nc.dram_tensor("ident", [128, 128], bf16, kind="ExternalInput").ap()
    outT = nc.dram_tensor("outT", [E, S], f32, kind="ExternalOutput").ap()

    busy = {"S": 0.0, "D": 0.0, "P": 0.0}
    exp_eng = _exp_engine_map()

    with tile.TileContext(nc) as tc:
        with tc.tile_pool(name="persist", bufs=1) as pers:
            # q-projection weights/bias first: they gate the very first
            # matmuls, so don't queue them behind the other ~1MB of DMAs
            wq_sb = pers.tile([128, 6, CL], bf16)
            nc.sync.dma_start(wq_sb[:], wqT.rearrange("(t p) c -> p t c", p=128))
            bq_sb = pers.tile([128, 2], f32)
            nc.sync.dma_start(bq_sb[:, 0:1], bq[0:128, :])
            nc.sync.dma_start(bq_sb[0:64, 1:2], bq[128:CL, :])

            wk_sb = pers.tile([128, 6, CL], bf16)
            wv_sb = pers.tile([128, 6, CL], bf16)
            wo_sb = pers.tile([128, 2, E], bf16)
            bk_sb = pers.tile([128, 2], f32)
            bvb_sb = pers.tile([128, CL], f32)
            kpT_sb = pers.tile([128, 2, PP], bf16)
            vp_sb = pers.tile([96, HL, D + 1], bf16)
            id_sb = pers.tile([128, 128], bf16)

            # activations (all bf16)
            qT_sb = pers.tile([128, 2, S], bf16)
            kT_sb = pers.tile([128, 2, S], bf16)   # no prefix; kpT separate
            v_sb = pers.tile([128, NST, HL, D + 1], bf16)
            ctxT_sb = pers.tile([128, 2, S], bf16)
            # prefix exp rows: head h at partitions 32h..32h+15
            # (gap rows zeroed once so the full-contraction prefix-ctx
            #  matmul contracts them harmlessly)
            expp_sb = pers.tile([96, S], bf16)

            nc.vector.memset(v_sb[:, :, :, D:D + 1], 1.0)
            nc.vector.memset(expp_sb[:], 0.0)

            # ---------------- Phase 1a: Q / K projections ----------------
            with (
                tc.tile_pool(name="ps_proj", bufs=2, space="PSUM") as pp,
                tc.tile_pool(name="xq_pool", bufs=4) as xq_pool,
            ):
                def proj_block(xin, wsb, bsb, dst, sq, skip_p1=False):
                    p0 = pp.tile([128, QT], f32, tag="p0", name="p0")
                    if not skip_p1:
                        p1 = pp.tile([64, QT], f32, tag="p1", name="p1")
                    for ech in range(6):
                        xt = xq_pool.tile([128, QT], bf16, tag="xt",
                                          name="xt")
                        nc.sync.dma_start(
                            xt[:],
                            xin[ech * 128:(ech + 1) * 128,
                                sq * QT:(sq + 1) * QT],
                        )
                        for n in range(QT // 512):
                            ns = slice(n * 512, (n + 1) * 512)
                            nc.tensor.matmul(
                                p0[:, ns], wsb[:, ech, 0:128], xt[:, ns],
                                start=(ech == 0), stop=(ech == 5),
                            )
                            if not skip_p1:
                                nc.tensor.matmul(
                                    p1[:, ns], wsb[:, ech, 128:CL],
                                    xt[:, ns],
                                    start=(ech == 0), stop=(ech == 5),
                                )
                    ds = slice(sq * QT, (sq + 1) * QT)
                    nc.vector.tensor_scalar_add(
                        dst[:, 0, ds], p0[:], bsb[:, 0:1])
                    if not skip_p1:
                        nc.vector.tensor_scalar_add(
                            dst[0:64, 1, ds], p1[:], bsb[0:64, 1:2])

                proj_block(xqT, wq_sb, bq_sb, qT_sb, 0)
                # now that the critical q DMAs are queued, stream in the
                # remaining weights behind them
                nc.sync.dma_start(
                    wk_sb[:], wkT.rearrange("(t p) c -> p t c", p=128))
                nc.sync.dma_start(bk_sb[:, 0:1], bk[0:128, :])
                nc.sync.dma_start(bk_sb[0:64, 1:2], bk[128:CL, :])
                nc.sync.dma_start(kpT_sb[:], kpT[:])
                nc.sync.dma_start(
                    wv_sb[:], wvT.rearrange("(t p) c -> p t c", p=128))
                nc.sync.dma_start(bvb_sb[:], bv.to_broadcast((128, CL)))
                nc.sync.dma_start(vp_sb[:], vp3[:])
                nc.sync.dma_start(id_sb[:], ident[:])
                nc.sync.dma_start(wo_sb[:, 0, :], woT[0:128, :])
                nc.sync.dma_start(wo_sb[0:64, 1, :], woT[128:CL, :])

                # prompt-prefix scores+exp (per head, like the baseline) --
                # exp rows land at expp partitions 16h..16h+16.  Starts
                # ScalarE (and its one-time exp table load) early.
                def emit_prefix(sq, psum_pool, tag):
                    # all 3 heads' prefix scores in one [96, QT] psum via
                    # quadrant tile positions -> a single ScalarE exp.  Gap
                    # rows hold exp(stale-psum) (finite); the prefix-ctx
                    # contracts them against vp3's zero rows.
                    psp = psum_pool.tile([96, QT], f32, tag=tag, name="psp")
                    # zero first: gap rows must exp() to something finite
                    nc.vector.memset(psp[:], 0.0)
                    for h in range(HL):
                        pr, po = h // 2, 64 * (h % 2)
                        for n in range(QT // 512):
                            ns = slice(n * 512, (n + 1) * 512)
                            qs = slice(sq * QT + n * 512,
                                       sq * QT + (n + 1) * 512)
                            nc.tensor.matmul(
                                psp[32 * h:32 * h + PP, ns],
                                kpT_sb[po:po + 64, pr, :],
                                qT_sb[po:po + 64, pr, qs],
                                start=True, stop=True,
                                tile_position=(po, 32 * h),
                            )
                    nc.scalar.activation(
                        expp_sb[:, sq * QT:(sq + 1) * QT], psp[:],
                        EXP, scale=float(D) ** -0.5,
                    )
                    busy["S"] += _cS(QT)

                emit_prefix(0, pp, "p0")

                for sq in range(NSQ):
                    proj_block(xkT, wk_sb, bk_sb, kT_sb, sq, skip_p1=True)

            # ---------- attention + V-proj + out-proj: one slot stream ----
            with (
                tc.tile_pool(name="ps_s", bufs=2, space="PSUM") as ps_s,
                tc.tile_pool(name="ps_c", bufs=1, space="PSUM") as ps_c,
                tc.tile_pool(name="ps_sm", bufs=2, space="PSUM") as ps_sm,
                tc.tile_pool(name="expt_pool", bufs=22) as expt_pool,
                tc.tile_pool(name="dve_scr", bufs=1) as dve_scr,
                tc.tile_pool(name="pool_scr", bufs=1) as pool_scr,
                tc.tile_pool(name="ctxn_pool", bufs=10) as ctxn_pool,
                tc.tile_pool(name="rc_pool", bufs=2) as rc_pool,
                tc.tile_pool(name="xv_pool", bufs=8) as xv_pool,
                tc.tile_pool(name="xq2_pool", bufs=12) as xq2_pool,
                tc.tile_pool(name="out_pool", bufs=4) as out_pool,
            ):
                def pick(cands):
                    """cands: {eng: cost_ns} -> engine with min projected busy."""
                    e = min(cands, key=lambda k: busy[k] + cands[k])
                    busy[e] += cands[e]
                    return e

                # Background q-projection for sq 1..3 (op-granular, drained
                # one op per stream slot using the time-multiplexed sm pool)
                def make_bg_qproj(sq):
                    ops = []
                    state = {}

                    def dma_op():
                        tiles = []
                        for ech in range(6):
                            xt2 = xq2_pool.tile([128, QT], bf16, tag="xt2",
                                                name="xt2")
                            nc.sync.dma_start(
                                xt2[:],
                                xqT[ech * 128:(ech + 1) * 128,
                                    sq * QT:(sq + 1) * QT],
                            )
                            tiles.append(xt2)
                        state["xt"] = tiles

                    ops.append(dma_op)

                    def mk_group(c, grp):
                        def op():
                            pt = ps_sm.tile([128, 512], f32, tag="sm",
                                            name="pq")
                            rows = 128 if grp == 0 else 64
                            wc = slice(0, 128) if grp == 0 else slice(128, CL)
                            for ech in range(6):
                                nc.tensor.matmul(
                                    pt[0:rows, :], wq_sb[:, ech, wc],
                                    state["xt"][ech][:, c * 512:(c + 1) * 512],
                                    start=(ech == 0), stop=(ech == 5),
                                )
                            qs = slice(sq * QT + c * 512,
                                       sq * QT + (c + 1) * 512)
                            busy["D"] += _cD(512)
                            if grp == 0:
                                dst, srcp, bias = (qT_sb[:, 0, qs], pt[:, :],
                                                   bq_sb[:, 0:1])
                            else:
                                dst, srcp, bias = (qT_sb[0:64, 1, qs],
                                                   pt[0:64, :],
                                                   bq_sb[0:64, 1:2])
                            nc.vector.tensor_scalar_add(dst, srcp, bias)
                        return op

                    for c in range(QT // 512):
                        for grp in range(2):
                            ops.append(mk_group(c, grp))
                    ops.append(lambda: emit_prefix(sq, ps_s, "pss"))
                    return ops

                kp1_tiles = {}

                def mk_kp1_dma(bsq):
                    def op():
                        tiles = []
                        for ech in range(6):
                            xt2 = xq2_pool.tile([128, QT], bf16, tag="xt2",
                                                name="xkp")
                            nc.sync.dma_start(
                                xt2[:],
                                xkT[ech * 128:(ech + 1) * 128,
                                    bsq * QT:(bsq + 1) * QT])
                            tiles.append(xt2)
                        kp1_tiles[bsq] = tiles
                    return op

                def mk_kp1_mm(bsq):
                    def op():
                        tiles = kp1_tiles.pop(bsq)
                        pk = ps_c.tile([64, QT], f32, tag="psc", name="pk")
                        for ech in range(6):
                            for n in range(QT // 512):
                                ns = slice(n * 512, (n + 1) * 512)
                                nc.tensor.matmul(
                                    pk[:, ns], wk_sb[:, ech, 128:CL],
                                    tiles[ech][:, ns],
                                    start=(ech == 0), stop=(ech == 5))
                        nc.vector.tensor_scalar_add(
                            kT_sb[0:64, 1, bsq * QT:(bsq + 1) * QT],
                            pk[:], bk_sb[0:64, 1:2])
                    return op

                bg_work = [(0, mk_kp1_dma(0))]
                for b in range(NSQ):
                    bg_work.append((3 * b + 3, mk_kp1_mm(b)))
                    if b + 1 < NSQ:
                        bg_work.append((3 * b + 3, mk_kp1_dma(b + 1)))
                for nb, sqb in ((32, 1), (70, 2), (150, 3)):
                    for op in make_bg_qproj(sqb):
                        bg_work.append((nb, op))

                # xv DMA loads, one sq-group of 6 chunks at a time
                xvts = {}

                def load_xv(sqx):
                    tiles = []
                    for ech in range(6):
                        xvt = xv_pool.tile([128, QT], bf16, tag="xvt",
                                           name="xvt")
                        nc.sync.dma_start(
                            xvt[:],
                            xvT[ech * 128:(ech + 1) * 128,
                                sqx * QT:(sqx + 1) * QT],
                        )
                        tiles.append(xvt)
                    xvts[sqx] = tiles

                def emit_vproj(st):
                    sqx, stl = st // (QT // 128), st % (QT // 128)
                    if st == 0:
                        load_xv(0)
                    if stl == 0 and sqx + 1 < NSQ:
                        load_xv(sqx + 1)
                    pv = ps_sm.tile([128, 512], f32, tag="sm", name="pv")
                    for ech in range(6):
                        nc.tensor.matmul(
                            pv[:, 0:CL],
                            xvts[sqx][ech][:, stl * 128:(stl + 1) * 128],
                            wv_sb[:, ech, :],
                            start=(ech == 0), stop=(ech == 5),
                        )
                    busy["D"] += _cD(CL)
                    nc.vector.tensor_add(
                        v_sb[:, st, :, 0:D],
                        pv[:, 0:CL].rearrange("p (h d) -> p h d", h=HL),
                        bvb_sb[:].rearrange("p (h d) -> p h d", h=HL),
                    )
                    if stl == (QT // 128) - 1:
                        del xvts[sqx]

                def emit_scores_exp(sq, h, kt, slot_idx):
                    pr, po = h // 2, 64 * (h % 2)
                    lhsT_k = kT_sb[po:po + 64, pr, kt * 128:(kt + 1) * 128]
                    pss = ps_s.tile([128, QT], f32, tag="pss", name="pss")
                    expt = expt_pool.tile([128, QT], bf16, tag="expt",
                                          name="expt")
                    for n in range(QT // 512):
                        ns = slice(n * 512, (n + 1) * 512)
                        qs = slice(sq * QT + n * 512, sq * QT + (n + 1) * 512)
                        nc.tensor.matmul(
                            pss[:, ns], lhsT_k, qT_sb[po:po + 64, pr, qs],
                            start=True, stop=True,
                        )
                    e = exp_eng[slot_idx]
                    if e == "S":
                        nc.scalar.activation(
                            expt[:], pss[:], EXP, scale=float(D) ** -0.5,
                        )
                        busy["S"] += _cS(QT)
                    elif e == "D":
                        # exp(s/8) ~ (1 + u + u^2/2)^8, u = s/64.
                        # op1 runs inline (it frees the scores psum slot);
                        # the 5 tail ops are spread via the flex queue so
                        # DVE's in-order queue stays shallow and never holds
                        # the scores ring hostage.
                        u = dve_scr.tile([128, QT], bf16, tag="du",
                                         name="du", bufs=2)
                        nc.vector.tensor_scalar(
                            u[:], pss[:], 1.0 / 64.0, None, ALU.mult)
                        a = dve_scr.tile([128, QT], bf16, tag="da",
                                         name="da", bufs=2)
                        t = dve_scr.tile([128, QT], bf16, tag="dt",
                                         name="dt", bufs=2)
                        s1 = dve_scr.tile([128, QT], bf16, tag="da",
                                          name="s1", bufs=2)
                        s2 = dve_scr.tile([128, QT], bf16, tag="dt",
                                          name="s2", bufs=2)
                        flex_work.append(lambda: nc.vector.scalar_tensor_tensor(
                            a[:], u[:], 0.5, u[:], ALU.mult, ALU.mult))
                        flex_work.append(lambda: nc.vector.scalar_tensor_tensor(
                            t[:], a[:], 1.0, u[:], ALU.add, ALU.add))
                        flex_work.append(lambda: nc.vector.tensor_mul(
                            s1[:], t[:], t[:]))
                        flex_work.append(lambda: nc.vector.tensor_mul(
                            s2[:], s1[:], s1[:]))
                        flex_work.append(lambda: nc.vector.tensor_mul(
                            expt[:], s2[:], s2[:]))
                        busy["D"] += _cD(QT) + 5 * (QT * 0.26 + 60.0)
                    else:
                        # Pool can't read PSUM: DVE does op1, Pool the rest
                        u = pool_scr.tile([128, QT], bf16, tag="pu",
                                          name="pu", bufs=3)
                        nc.vector.tensor_scalar(
                            u[:], pss[:], 1.0 / 64.0, None, ALU.mult)
                        busy["D"] += _cD(QT)
                        a = pool_scr.tile([128, QT], bf16, tag="pa",
                                          name="pa", bufs=2)
                        btl = pool_scr.tile([128, QT], bf16, tag="pb",
                                            name="btl", bufs=2)
                        t = pool_scr.tile([128, QT], bf16, tag="pa",
                                          name="t2", bufs=2)
                        s1 = pool_scr.tile([128, QT], bf16, tag="pb",
                                           name="s1", bufs=2)
                        s2 = pool_scr.tile([128, QT], bf16, tag="pa",
                                           name="s2", bufs=2)
                        flex_work.append(lambda: nc.gpsimd.tensor_mul(
                            a[:], u[:], u[:]))
                        flex_work.append(lambda: nc.gpsimd.tensor_scalar(
                            btl[:], a[:], 0.5, 1.0, ALU.mult, ALU.add))
                        flex_work.append(lambda: nc.gpsimd.tensor_add(
                            t[:], btl[:], u[:]))
                        flex_work.append(lambda: nc.gpsimd.tensor_mul(
                            s1[:], t[:], t[:]))
                        flex_work.append(lambda: nc.gpsimd.tensor_mul(
                            s2[:], s1[:], s1[:]))
                        flex_work.append(lambda: nc.gpsimd.tensor_mul(
                            expt[:], s2[:], s2[:]))
                        busy["P"] += _cP(QT) + 5 * _cP(QT, 0.42)
                    return expt

                flex_work = []   # norm/transpose/copy chains, drained per slot
                psc_tiles = {}

                def emit_ctx(sq, h, kt, expt):
                    key = (sq, h)
                    if kt == 0:
                        psc_tiles[key] = ps_c.tile([128, NQC, 128], f32,
                                                   tag="psc", name="psc")
                    psc = psc_tiles[key]
                    for i in range(NQC):
                        # start=True resets the WHOLE psum bank, so only the
                        # first region of each bank (4 regions/bank) may set
                        # it; the bank-wide zero covers the other regions.
                        nc.tensor.matmul(
                            psc[:, i, 0:D + 1],
                            expt[:, i * 128:(i + 1) * 128],
                            v_sb[:, kt, h, :],
                            start=(kt == 0 and i % 4 == 0),
                            stop=(kt == NKT - 1),
                            skip_group_check=True,
                        )
                    if kt == TRAIL - 1:
                        # prompt-prefix ctx contribution: full 48-row
                        # contraction; other heads' expp rows hit vp3 zeros
                        for i in range(NQC):
                            qs = slice(sq * QT + i * 128,
                                       sq * QT + (i + 1) * 128)
                            nc.tensor.matmul(
                                psc[:, i, 0:D + 1],
                                expp_sb[:, qs],
                                vp_sb[:, h, :],
                                start=False, stop=False,
                            )
                    if kt == NKT - 1:
                        queue_norm(sq, h, psc)
                        del psc_tiles[key]

                def queue_norm(sq, h, psc):
                    # recip + the 8 normalize multiplies run inline so psc
                    # frees quickly (it is single-buffered); the transpose +
                    # ctxT copies are queued -- they only gate out-proj.
                    pr, po = h // 2, 64 * (h % 2)
                    rc = rc_pool.tile([128, NQC, 1], f32, tag="rc", name="rc")
                    nc.vector.reciprocal(rc[:], psc[:, :, D:D + 1])
                    busy["D"] += _cD(NQC)
                    ctxns = []
                    for i in range(NQC):
                        ctxn = ctxn_pool.tile([128, D], bf16, tag="cn",
                                              name="ctxn")
                        busy["D"] += _cD(D)
                        nc.vector.tensor_scalar_mul(
                            ctxn[:], psc[:, i, 0:D], rc[:, i, 0:1])
                        ctxns.append(ctxn)

                    def mk_chunk(i):
                        def op():
                            ptr = ps_sm.tile([64, 128], bf16, tag="sm",
                                             name="ptr")
                            nc.tensor.transpose(ptr[:], ctxns[i][:], id_sb[:])
                            qs = slice(sq * QT + i * 128,
                                       sq * QT + (i + 1) * 128)
                            busy["D"] += _cD(128)
                            nc.vector.tensor_copy(
                                ctxT_sb[po:po + 64, pr, qs], ptr[:])
                        return op

                    for i in range(NQC):
                        flex_work.append(mk_chunk(i))
                    if h == HL - 1:
                        flex_work.append(lambda: emit_outproj(sq))

                outproj_work = []

                def emit_outproj(sq):
                    # queue the 12 out-projection tiles; drained 1/slot so
                    # they never lump up in front of scores matmuls
                    for et in range(6):
                        for n in range(QT // 512):
                            outproj_work.append((et, sq * 2 + n))

                def emit_outproj_tile(et, qn):
                    es = slice(et * 128, (et + 1) * 128)
                    qs = slice(qn * 512, (qn + 1) * 512)
                    po3 = ps_sm.tile([128, 512], f32, tag="sm", name="po3")
                    nc.tensor.matmul(
                        po3[:], wo_sb[:, 0, es], ctxT_sb[:, 0, qs],
                        start=True, stop=False,
                    )
                    nc.tensor.matmul(
                        po3[:], wo_sb[0:64, 1, es], ctxT_sb[0:64, 1, qs],
                        start=False, stop=True,
                    )
                    ot = out_pool.tile([128, 512], f32, tag="ot", name="ot")
                    busy["D"] += _cD(512)
                    nc.vector.tensor_copy(ot[:], po3[:])
                    nc.sync.dma_start(outT[es, qs], ot[:])

                slots = [(sq, h, kt)
                         for sq in range(NSQ)
                         for h in range(HL)
                         for kt in range(NKT)]
                pending = []

                def pop_one():
                    (s2, e2) = pending.pop(0)
                    emit_ctx(*s2, e2)

                vst = 0
                for j, slot in enumerate(slots):
                    # scores matmuls first in each slot so the exp engines'
                    # feed is never queue-delayed
                    expt = emit_scores_exp(*slot, j)
                    pending.append((slot, expt))
                    if vst < NST:
                        emit_vproj(vst)
                        vst += 1
                    # near the stream end the trail no longer buys slack --
                    # drain it so the final norm/out-proj/store chain starts
                    # as early as possible
                    trail_eff = TRAIL if j < len(slots) - 40 else 2
                    for _ in range(3):
                        if not pending:
                            break
                        need = (trail_eff + GAP if pending[0][0][2] == 0
                                else trail_eff)
                        if len(pending) > need:
                            pop_one()
                        else:
                            break
                    # drain one background / outproj op per slot (they share
                    # the sm psum ring so never interleave bg with outproj)
                    if bg_work and j >= bg_work[0][0]:
                        bg_work.pop(0)[1]()
                    elif outproj_work:
                        emit_outproj_tile(*outproj_work.pop(0))
                    # flex ops (chain tails, transposes, copies): 2-3 per slot
                    nfl = 3 if len(flex_work) > 10 else 2
                    for _ in range(nfl):
                        if not flex_work:
                            break
                        flex_work.pop(0)()
                while pending:
                    pop_one()
                    for _ in range(3):
                        if flex_work:
                            flex_work.pop(0)()
                    if outproj_work:
                        emit_outproj_tile(*outproj_work.pop(0))
                for _, op in bg_work:
                    op()
                while flex_work:
                    flex_work.pop(0)()
                while outproj_work:
                    emit_outproj_tile(*outproj_work.pop(0))

    nc.compile()
    return nc


def _get_nc():
    with _lock:
        if "nc" not in _compiled:
            _compiled["nc"] = _build()
        return _compiled["nc"]


def _prep_in_maps(query, key, value, prompt, Wq, bq, Wk, bk, Wv, bv, Wo, bo):
    f32 = np.float32
    qT = [np.ascontiguousarray(query[b].T).astype(BF16) for b in range(B)]
    kT = [np.ascontiguousarray(key[b].T).astype(BF16) for b in range(B)]
    vT = [np.ascontiguousarray(value[b].T).astype(BF16) for b in range(B)]
    ident = np.eye(128, dtype=BF16)
    in_maps = []
    for core in range(NCORES):
        b, g = core // NG, core % NG
        cs = slice(g * CL, (g + 1) * CL)
        kp = np.zeros((128, 2, PP), BF16)
        vpa = np.zeros((96, HL, D + 1), BF16)
        for h in range(HL):
            gh = g * HL + h
            kp[64 * (h % 2):64 * (h % 2) + 64, h // 2, :] = (
                prompt[b, 0, :, gh, :].T.astype(BF16))
            vpa[32 * h:32 * h + PP, h, 0:D] = (
                prompt[b, 1, :, gh, :].astype(BF16))
            vpa[32 * h:32 * h + PP, h, D] = 1.0
        in_maps.append({
            "xqT": qT[b], "xkT": kT[b], "xvT": vT[b],
            "wqT": np.ascontiguousarray(Wq[cs, :].T).astype(BF16),
            "wkT": np.ascontiguousarray(Wk[cs, :].T).astype(BF16),
            "wvT": np.ascontiguousarray(Wv[cs, :].T).astype(BF16),
            "woT": np.ascontiguousarray(Wo[:, cs].T).astype(BF16),
            "bq": np.ascontiguousarray(bq[cs]).astype(f32).reshape(CL, 1),
            "bk": np.ascontiguousarray(bk[cs]).astype(f32).reshape(CL, 1),
            "bv": np.ascontiguousarray(bv[cs]).astype(f32).reshape(1, CL),
            "kpT": kp, "vp3": vpa, "ident": ident,
        })
    return in_maps


def _combine(results, bo):
    out = np.empty((B, S, E), np.float32)
    for b in range(B):
        acc = results[b * NG]["outT"].astype(np.float32)
        for g in range(1, NG):
            acc = acc + results[b * NG + g]["outT"]
        out[b] = acc.T
    if bo is not None and np.any(bo):
        out += np.asarray(bo, np.float32)
    return out


def run(inputs, trace=False):
    """Returns (output, exec_time_ns or None)."""
    from concourse import bass_utils

    nc = _get_nc()
    in_maps = _prep_in_maps(**{k: np.asarray(v) for k, v in inputs.items()})
    bo = np.asarray(inputs["bo"])
    res = bass_utils.run_bass_kernel_spmd(
        nc, in_maps, core_ids=list(range(NCORES)), trace=trace,
    )
    return _combine(res.results, bo), res.exec_time_ns


def kernel(**inputs):
    out, _ = run(inputs)
    return out


# revision 36
# speedup vs baseline: 1.0150x; 1.0150x over previous
"""Multi-head attention (B=2,S=4096,E=768,H=12,D=64 + 16-token K/V prompt
prefix) on 8 Trainium2 NeuronCores.

Sharding: 2 batches x 4 head-groups (3 heads each). Each core computes QKV
projections for its 3 heads, full attention over its batch, and a partial
output projection (its 192 ctx channels); the host sums the 4 partials per
batch.

Per-core kernel layout (cost model: matmul = out-free-size rows; activation/
DVE/Pool = free-size * cycle):
  qT[c,s]   = Wq_g @ query^T            (lhsT=Wq_g^T chunks, rhs=queryT chunks)
  kT[c,s]   likewise; prompt K prefix kept separate (kpT)
  v[s,c]    natural orientation          (lhsT=valueT chunks, rhs=Wv_g^T)
  scoresT[k,q] = kT-tile^T @ qT          (lhsT=kT tile [64,128], rhs=qT [64,512])
  expT = Exp(scoresT / 8)                (ScalarE, or DVE/Pool via the
                                          (1+u+u^2/2)^8 chain, u=s/64)
  ctx[q, d+denom] += expT-slice^T @ v    (STATIONARY-SWAPPED: lhsT=expT
                                          [128kv,128q], rhs=v_aug [128kv,65]
                                          -> only 65 pumped rows per k-tile
                                          per q-chunk, half the PE cost of the
                                          [d,q] orientation)
  ctx_n[q,c] = ctx * recip(denom[q])     (per-partition scalar multiply)
  ctxT[c,q] = PE-transpose(ctx_n)        (via identity; rides scores psum ring)
  outT[e,q] partial = Wo_g^T-slices @ ctxT

Exp tiles are statically split across ScalarE / DVE / Pool so every engine's
modeled busy time lands near the PE's ~340us. Flexible evacuation ops
(normalize, psum->sbuf copies, out-proj evac) go through a greedy busy-time
balancer across DVE/Pool/ScalarE.
"""

import sys
import threading

import numpy as np

if "/opt/trn_rl_repo" not in sys.path:
    sys.path.insert(0, "/opt/trn_rl_repo")

import ml_dtypes

BF16 = ml_dtypes.bfloat16

B, S, E, H, D, PP = 2, 4096, 768, 12, 64, 16
NCORES = 8
NG = 4          # head-groups (tensor parallel)
HL = H // NG    # 3 local heads
CL = HL * D     # 192 local channels
SKV = PP + S    # 4112
NKT = S // 128  # 32 full k-tiles (prefix handled separately)
QT = 1024       # q tile width for scores/exp
NSQ = S // QT   # 4
NQC = QT // 128  # 8 q-chunks of 128 within a q tile
TRAIL = 16      # ctx matmuls trail scores by this many slots
GAP = 6         # extra trail for block-opening ctx (psc single-buffered)
NST = S // 128  # 32 v stiles

# --- engine split knobs ----------------------------------------------------
# (Pool/GPSIMD cannot access PSUM, so Pool-assigned exp tiles have their
#  first op (psum read) done by DVE; Pool runs the 6 SBUF-only chain ops.)
N_DVE_EXP = 0    # exp tiles handled end-to-end by the DVE 6-op chain
N_POOL_EXP = 0   # exp tiles: DVE op1 + Pool 6-op SBUF tail
# (Measured: every chain-offloaded tile ends up costing ~2.5us of scores-ring
#  stall -- the chain's first op queues behind DVE's in-order backlog and the
#  2-slot scores psum ring serializes on first-reader completion.  ScalarE-only
#  exp is the fastest measured configuration; the ctx stationary-swap keeps PE
#  ~80us below the baseline so ScalarE stays the sole pacer.)

# modeled per-op busy-time (ns) helpers for the greedy balancer
def _cS(f):       # ScalarE activation
    return (f + 222) * 0.8333

def _cD(f):       # DVE tensor op (psum involved)
    return f * 1.0417 + 125.0

def _cP(f, eff=0.6):  # Pool tensor op
    return f * 0.8333 / eff + 95.0

_lock = threading.Lock()
_compiled = {}


def _exp_engine_map():
    """Static slot -> engine ('S'/'D'/'P') assignment for the 384 exp tiles."""
    n_slots = NSQ * HL * NKT
    eng = ["S"] * n_slots
    # Pool tiles: long latency chain (13+ slots) -> keep away from stream ends
    for i in range(N_POOL_EXP):
        s = 40 + int(i * (n_slots - 108) / max(N_POOL_EXP, 1))
        eng[s] = "P"
    for i in range(N_DVE_EXP):
        s = 36 + int(i * (n_slots - 80) / max(N_DVE_EXP, 1))
        while eng[s] != "S":
            s += 1
        eng[s] = "D"
    return eng


def _build():
    import concourse.bass as bass  # noqa: F401
    import concourse.mybir as mybir
    import concourse.tile as tile
    from concourse import bacc

    f32 = mybir.dt.float32
    bf16 = mybir.dt.bfloat16
    EXP = mybir.ActivationFunctionType.Exp
    ALU = mybir.AluOpType

    nc = bacc.Bacc("TRN2", target_bir_lowering=False, debug=False)

    xqT = nc.dram_tensor("xqT", [E, S], bf16, kind="ExternalInput").ap()
    xkT = nc.dram_tensor("xkT", [E, S], bf16, kind="ExternalInput").ap()
    xvT = nc.dram_tensor("xvT", [E, S], bf16, kind="ExternalInput").ap()
    wqT = nc.dram_tensor("wqT", [E, CL], bf16, kind="ExternalInput").ap()
    wkT = nc.dram_tensor("wkT", [E, CL], bf16, kind="ExternalInput").ap()
    wvT = nc.dram_tensor("wvT", [E, CL], bf16, kind="ExternalInput").ap()
    woT = nc.dram_tensor("woT", [CL, E], bf16, kind="ExternalInput").ap()
    bq = nc.dram_tensor("bq", [CL, 1], f32, kind="ExternalInput").ap()
    bk = nc.dram_tensor("bk", [CL, 1], f32, kind="ExternalInput").ap()
    bv = nc.dram_tensor("bv", [1, CL], f32, kind="ExternalInput").ap()
    kpT = nc.dram_tensor("kpT", [128, 2, PP], bf16, kind="ExternalInput").ap()
    # vp3[:, h, :]: head h prefix V at rows 16h..16h+16 (zeros elsewhere),
    # col 64 = ones on those rows.  Zero-padding makes the prefix-ctx matmul
    # a plain full-contraction matmul (other heads' expp rows hit zeros).
    vp3 = nc.dram_tensor("vp3", [96, HL, D + 1], bf16,
                         kind="ExternalInput").ap()
    ident = nc.dram_tensor("ident", [128, 128], bf16, kind="ExternalInput").ap()
    outT = nc.dram_tensor("outT", [E, S], f32, kind="ExternalOutput").ap()

    busy = {"S": 0.0, "D": 0.0, "P": 0.0}
    exp_eng = _exp_engine_map()

    with tile.TileContext(nc) as tc:
        with tc.tile_pool(name="persist", bufs=1) as pers:
            # q-projection weights/bias first: they gate the very first
            # matmuls, so don't queue them behind the other ~1MB of DMAs
            wq_sb = pers.tile([128, 6, CL], bf16)
            nc.sync.dma_start(wq_sb[:], wqT.rearrange("(t p) c -> p t c", p=128))
            bq_sb = pers.tile([128, 2], f32)
            nc.sync.dma_start(bq_sb[:, 0:1], bq[0:128, :])
            nc.sync.dma_start(bq_sb[0:64, 1:2], bq[128:CL, :])

            wk_sb = pers.tile([128, 6, CL], bf16)
            wv_sb = pers.tile([128, 6, CL], bf16)
            wo_sb = pers.tile([128, 2, E], bf16)
            bk_sb = pers.tile([128, 2], f32)
            bvb_sb = pers.tile([128, CL], f32)
            kpT_sb = pers.tile([128, 2, PP], bf16)
            vp_sb = pers.tile([96, HL, D + 1], bf16)
            id_sb = pers.tile([128, 128], bf16)

            # activations (all bf16)
            qT_sb = pers.tile([128, 2, S], bf16)
            kT_sb = pers.tile([128, 2, S], bf16)   # no prefix; kpT separate
            v_sb = pers.tile([128, NST, HL, D + 1], bf16)
            ctxT_sb = pers.tile([128, 2, S], bf16)
            # prefix exp rows: head h at partitions 32h..32h+15
            # (gap rows zeroed once so the full-contraction prefix-ctx
            #  matmul contracts them harmlessly)
            expp_sb = pers.tile([96, S], bf16)

            nc.vector.memset(v_sb[:, :, :, D:D + 1], 1.0)
            nc.vector.memset(expp_sb[:], 0.0)

            # ---------------- Phase 1a: Q / K projections ----------------
            with (
                tc.tile_pool(name="ps_proj", bufs=2, space="PSUM") as pp,
                tc.tile_pool(name="xq_pool", bufs=4) as xq_pool,
            ):
                def proj_block(xin, wsb, bsb, dst, sq, skip_p1=False):
                    p0 = pp.tile([128, QT], f32, tag="p0", name="p0")
                    if not skip_p1:
                        p1 = pp.tile([64, QT], f32, tag="p1", name="p1")
                    for ech in range(6):
                        xt = xq_pool.tile([128, QT], bf16, tag="xt",
                                          name="xt")
                        nc.sync.dma_start(
                            xt[:],
                            xin[ech * 128:(ech + 1) * 128,
                                sq * QT:(sq + 1) * QT],
                        )
                        for n in range(QT // 512):
                            ns = slice(n * 512, (n + 1) * 512)
                            nc.tensor.matmul(
                                p0[:, ns], wsb[:, ech, 0:128], xt[:, ns],
                                start=(ech == 0), stop=(ech == 5),
                            )
                            if not skip_p1:
                                nc.tensor.matmul(
                                    p1[:, ns], wsb[:, ech, 128:CL],
                                    xt[:, ns],
                                    start=(ech == 0), stop=(ech == 5),
                                )
                    ds = slice(sq * QT, (sq + 1) * QT)
                    nc.vector.tensor_scalar_add(
                        dst[:, 0, ds], p0[:], bsb[:, 0:1])
                    if not skip_p1:
                        nc.vector.tensor_scalar_add(
                            dst[0:64, 1, ds], p1[:], bsb[0:64, 1:2])

                proj_block(xqT, wq_sb, bq_sb, qT_sb, 0)
                # now that the critical q DMAs are queued, stream in the
                # remaining weights behind them
                nc.sync.dma_start(
                    wk_sb[:], wkT.rearrange("(t p) c -> p t c", p=128))
                nc.sync.dma_start(bk_sb[:, 0:1], bk[0:128, :])
                nc.sync.dma_start(bk_sb[0:64, 1:2], bk[128:CL, :])
                nc.sync.dma_start(kpT_sb[:], kpT[:])
                nc.sync.dma_start(
                    wv_sb[:], wvT.rearrange("(t p) c -> p t c", p=128))
                nc.sync.dma_start(bvb_sb[:], bv.to_broadcast((128, CL)))
                nc.sync.dma_start(vp_sb[:], vp3[:])
                nc.sync.dma_start(id_sb[:], ident[:])
                nc.sync.dma_start(wo_sb[:, 0, :], woT[0:128, :])
                nc.sync.dma_start(wo_sb[0:64, 1, :], woT[128:CL, :])

                # prompt-prefix scores+exp (per head, like the baseline) --
                # exp rows land at expp partitions 16h..16h+16.  Starts
                # ScalarE (and its one-time exp table load) early.
                def emit_prefix(sq, psum_pool, tag):
                    # all 3 heads' prefix scores in one [96, QT] psum via
                    # quadrant tile positions -> a single ScalarE exp.  Gap
                    # rows hold exp(stale-psum) (finite); the prefix-ctx
                    # contracts them against vp3's zero rows.
                    psp = psum_pool.tile([96, QT], f32, tag=tag, name="psp")
                    # zero first: gap rows must exp() to something finite
                    nc.vector.memset(psp[:], 0.0)
                    for h in range(HL):
                        pr, po = h // 2, 64 * (h % 2)
                        for n in range(QT // 512):
                            ns = slice(n * 512, (n + 1) * 512)
                            qs = slice(sq * QT + n * 512,
                                       sq * QT + (n + 1) * 512)
                            nc.tensor.matmul(
                                psp[32 * h:32 * h + PP, ns],
                                kpT_sb[po:po + 64, pr, :],
                                qT_sb[po:po + 64, pr, qs],
                                start=True, stop=True,
                                tile_position=(po, 32 * h),
                            )
                    nc.scalar.activation(
                        expp_sb[:, sq * QT:(sq + 1) * QT], psp[:],
                        EXP, scale=float(D) ** -0.5,
                    )
                    busy["S"] += _cS(QT)

                emit_prefix(0, pp, "p0")

                for sq in range(NSQ):
                    proj_block(xkT, wk_sb, bk_sb, kT_sb, sq)

            # ---------- attention + V-proj + out-proj: one slot stream ----
            with (
                tc.tile_pool(name="ps_s", bufs=2, space="PSUM") as ps_s,
                tc.tile_pool(name="ps_c", bufs=1, space="PSUM") as ps_c,
                tc.tile_pool(name="ps_sm", bufs=2, space="PSUM") as ps_sm,
                tc.tile_pool(name="expt_pool", bufs=22) as expt_pool,
                tc.tile_pool(name="dve_scr", bufs=1) as dve_scr,
                tc.tile_pool(name="pool_scr", bufs=1) as pool_scr,
                tc.tile_pool(name="ctxn_pool", bufs=10) as ctxn_pool,
                tc.tile_pool(name="rc_pool", bufs=2) as rc_pool,
                tc.tile_pool(name="xv_pool", bufs=8) as xv_pool,
                tc.tile_pool(name="xq2_pool", bufs=6) as xq2_pool,
                tc.tile_pool(name="out_pool", bufs=4) as out_pool,
            ):
                def pick(cands):
                    """cands: {eng: cost_ns} -> engine with min projected busy."""
                    e = min(cands, key=lambda k: busy[k] + cands[k])
                    busy[e] += cands[e]
                    return e

                # Background q-projection for sq 1..3 (op-granular, drained
                # one op per stream slot using the time-multiplexed sm pool)
                def make_bg_qproj(sq):
                    ops = []
                    state = {}

                    def dma_op():
                        tiles = []
                        for ech in range(6):
                            xt2 = xq2_pool.tile([128, QT], bf16, tag="xt2",
                                                name="xt2")
                            nc.sync.dma_start(
                                xt2[:],
                                xqT[ech * 128:(ech + 1) * 128,
                                    sq * QT:(sq + 1) * QT],
                            )
                            tiles.append(xt2)
                        state["xt"] = tiles

                    ops.append(dma_op)

                    def mk_group(c, grp):
                        def op():
                            pt = ps_sm.tile([128, 512], f32, tag="sm",
                                            name="pq")
                            rows = 128 if grp == 0 else 64
                            wc = slice(0, 128) if grp == 0 else slice(128, CL)
                            for ech in range(6):
                                nc.tensor.matmul(
                                    pt[0:rows, :], wq_sb[:, ech, wc],
                                    state["xt"][ech][:, c * 512:(c + 1) * 512],
                                    start=(ech == 0), stop=(ech == 5),
                                )
                            qs = slice(sq * QT + c * 512,
                                       sq * QT + (c + 1) * 512)
                            busy["D"] += _cD(512)
                            if grp == 0:
                                dst, srcp, bias = (qT_sb[:, 0, qs], pt[:, :],
                                                   bq_sb[:, 0:1])
                            else:
                                dst, srcp, bias = (qT_sb[0:64, 1, qs],
                                                   pt[0:64, :],
                                                   bq_sb[0:64, 1:2])
                            nc.vector.tensor_scalar_add(dst, srcp, bias)
                        return op

                    for c in range(QT // 512):
                        for grp in range(2):
                            ops.append(mk_group(c, grp))
                    ops.append(lambda: emit_prefix(sq, ps_s, "pss"))
                    return ops

                bg_work = []
                for nb, sqb in ((32, 1), (70, 2), (150, 3)):
                    for op in make_bg_qproj(sqb):
                        bg_work.append((nb, op))

                # xv DMA loads, one sq-group of 6 chunks at a time
                xvts = {}

                def load_xv(sqx):
                    tiles = []
                    for ech in range(6):
                        xvt = xv_pool.tile([128, QT], bf16, tag="xvt",
                                           name="xvt")
                        nc.sync.dma_start(
                            xvt[:],
                            xvT[ech * 128:(ech + 1) * 128,
                                sqx * QT:(sqx + 1) * QT],
                        )
                        tiles.append(xvt)
                    xvts[sqx] = tiles

                def emit_vproj(st):
                    sqx, stl = st // (QT // 128), st % (QT // 128)
                    if st == 0:
                        load_xv(0)
                    if stl == 0 and sqx + 1 < NSQ:
                        load_xv(sqx + 1)
                    pv = ps_sm.tile([128, 512], f32, tag="sm", name="pv")
                    for ech in range(6):
                        nc.tensor.matmul(
                            pv[:, 0:CL],
                            xvts[sqx][ech][:, stl * 128:(stl + 1) * 128],
                            wv_sb[:, ech, :],
                            start=(ech == 0), stop=(ech == 5),
                        )
                    busy["D"] += _cD(CL)
                    nc.vector.tensor_add(
                        v_sb[:, st, :, 0:D],
                        pv[:, 0:CL].rearrange("p (h d) -> p h d", h=HL),
                        bvb_sb[:].rearrange("p (h d) -> p h d", h=HL),
                    )
                    if stl == (QT // 128) - 1:
                        del xvts[sqx]

                def emit_scores_exp(sq, h, kt, slot_idx):
                    pr, po = h // 2, 64 * (h % 2)
                    lhsT_k = kT_sb[po:po + 64, pr, kt * 128:(kt + 1) * 128]
                    pss = ps_s.tile([128, QT], f32, tag="pss", name="pss")
                    expt = expt_pool.tile([128, QT], bf16, tag="expt",
                                          name="expt")
                    for n in range(QT // 512):
                        ns = slice(n * 512, (n + 1) * 512)
                        qs = slice(sq * QT + n * 512, sq * QT + (n + 1) * 512)
                        nc.tensor.matmul(
                            pss[:, ns], lhsT_k, qT_sb[po:po + 64, pr, qs],
                            start=True, stop=True,
                        )
                    e = exp_eng[slot_idx]
                    if e == "S":
                        nc.scalar.activation(
                            expt[:], pss[:], EXP, scale=float(D) ** -0.5,
                        )
                        busy["S"] += _cS(QT)
                    elif e == "D":
                        # exp(s/8) ~ (1 + u + u^2/2)^8, u = s/64.
                        # op1 runs inline (it frees the scores psum slot);
                        # the 5 tail ops are spread via the flex queue so
                        # DVE's in-order queue stays shallow and never holds
                        # the scores ring hostage.
                        u = dve_scr.tile([128, QT], bf16, tag="du",
                                         name="du", bufs=2)
                        nc.vector.tensor_scalar(
                            u[:], pss[:], 1.0 / 64.0, None, ALU.mult)
                        a = dve_scr.tile([128, QT], bf16, tag="da",
                                         name="da", bufs=2)
                        t = dve_scr.tile([128, QT], bf16, tag="dt",
                                         name="dt", bufs=2)
                        s1 = dve_scr.tile([128, QT], bf16, tag="da",
                                          name="s1", bufs=2)
                        s2 = dve_scr.tile([128, QT], bf16, tag="dt",
                                          name="s2", bufs=2)
                        flex_work.append(lambda: nc.vector.scalar_tensor_tensor(
                            a[:], u[:], 0.5, u[:], ALU.mult, ALU.mult))
                        flex_work.append(lambda: nc.vector.scalar_tensor_tensor(
                            t[:], a[:], 1.0, u[:], ALU.add, ALU.add))
                        flex_work.append(lambda: nc.vector.tensor_mul(
                            s1[:], t[:], t[:]))
                        flex_work.append(lambda: nc.vector.tensor_mul(
                            s2[:], s1[:], s1[:]))
                        flex_work.append(lambda: nc.vector.tensor_mul(
                            expt[:], s2[:], s2[:]))
                        busy["D"] += _cD(QT) + 5 * (QT * 0.26 + 60.0)
                    else:
                        # Pool can't read PSUM: DVE does op1, Pool the rest
                        u = pool_scr.tile([128, QT], bf16, tag="pu",
                                          name="pu", bufs=3)
                        nc.vector.tensor_scalar(
                            u[:], pss[:], 1.0 / 64.0, None, ALU.mult)
                        busy["D"] += _cD(QT)
                        a = pool_scr.tile([128, QT], bf16, tag="pa",
                                          name="pa", bufs=2)
                        btl = pool_scr.tile([128, QT], bf16, tag="pb",
                                            name="btl", bufs=2)
                        t = pool_scr.tile([128, QT], bf16, tag="pa",
                                          name="t2", bufs=2)
                        s1 = pool_scr.tile([128, QT], bf16, tag="pb",
                                           name="s1", bufs=2)
                        s2 = pool_scr.tile([128, QT], bf16, tag="pa",
                                           name="s2", bufs=2)
                        flex_work.append(lambda: nc.gpsimd.tensor_mul(
                            a[:], u[:], u[:]))
                        flex_work.append(lambda: nc.gpsimd.tensor_scalar(
                            btl[:], a[:], 0.5, 1.0, ALU.mult, ALU.add))
                        flex_work.append(lambda: nc.gpsimd.tensor_add(
                            t[:], btl[:], u[:]))
                        flex_work.append(lambda: nc.gpsimd.tensor_mul(
                            s1[:], t[:], t[:]))
                        flex_work.append(lambda: nc.gpsimd.tensor_mul(
                            s2[:], s1[:], s1[:]))
                        flex_work.append(lambda: nc.gpsimd.tensor_mul(
                            expt[:], s2[:], s2[:]))
                        busy["P"] += _cP(QT) + 5 * _cP(QT, 0.42)
                    return expt

                flex_work = []   # norm/transpose/copy chains, drained per slot
                psc_tiles = {}

                def emit_ctx(sq, h, kt, expt):
                    key = (sq, h)
                    if kt == 0:
                        psc_tiles[key] = ps_c.tile([128, NQC, 128], f32,
                                                   tag="psc", name="psc")
                    psc = psc_tiles[key]
                    for i in range(NQC):
                        # start=True resets the WHOLE psum bank, so only the
                        # first region of each bank (4 regions/bank) may set
                        # it; the bank-wide zero covers the other regions.
                        nc.tensor.matmul(
                            psc[:, i, 0:D + 1],
                            expt[:, i * 128:(i + 1) * 128],
                            v_sb[:, kt, h, :],
                            start=(kt == 0 and i % 4 == 0),
                            stop=(kt == NKT - 1),
                            skip_group_check=True,
                        )
                    if kt == TRAIL - 1:
                        # prompt-prefix ctx contribution: full 48-row
                        # contraction; other heads' expp rows hit vp3 zeros
                        for i in range(NQC):
                            qs = slice(sq * QT + i * 128,
                                       sq * QT + (i + 1) * 128)
                            nc.tensor.matmul(
                                psc[:, i, 0:D + 1],
                                expp_sb[:, qs],
                                vp_sb[:, h, :],
                                start=False, stop=False,
                            )
                    if kt == NKT - 1:
                        queue_norm(sq, h, psc)
                        del psc_tiles[key]

                def queue_norm(sq, h, psc):
                    # recip + the 8 normalize multiplies run inline so psc
                    # frees quickly (it is single-buffered); the transpose +
                    # ctxT copies are queued -- they only gate out-proj.
                    pr, po = h // 2, 64 * (h % 2)
                    rc = rc_pool.tile([128, NQC, 1], f32, tag="rc", name="rc")
                    nc.vector.reciprocal(rc[:], psc[:, :, D:D + 1])
                    busy["D"] += _cD(NQC)
                    ctxns = []
                    for i in range(NQC):
                        ctxn = ctxn_pool.tile([128, D], bf16, tag="cn",
                                              name="ctxn")
                        busy["D"] += _cD(D)
                        nc.vector.tensor_scalar_mul(
                            ctxn[:], psc[:, i, 0:D], rc[:, i, 0:1])
                        ctxns.append(ctxn)

                    def mk_chunk(i):
                        def op():
                            ptr = ps_sm.tile([64, 128], bf16, tag="sm",
                                             name="ptr")
                            nc.tensor.transpose(ptr[:], ctxns[i][:], id_sb[:])
                            qs = slice(sq * QT + i * 128,
                                       sq * QT + (i + 1) * 128)
                            busy["D"] += _cD(128)
                            nc.vector.tensor_copy(
                                ctxT_sb[po:po + 64, pr, qs], ptr[:])
                        return op

                    for i in range(NQC):
                        flex_work.append(mk_chunk(i))
                    if h == HL - 1:
                        flex_work.append(lambda: emit_outproj(sq))

                outproj_work = []

                def emit_outproj(sq):
                    # queue the 12 out-projection tiles; drained 1/slot so
                    # they never lump up in front of scores matmuls
                    for et in range(6):
                        for n in range(QT // 512):
                            outproj_work.append((et, sq * 2 + n))

                def emit_outproj_tile(et, qn):
                    es = slice(et * 128, (et + 1) * 128)
                    qs = slice(qn * 512, (qn + 1) * 512)
                    po3 = ps_sm.tile([128, 512], f32, tag="sm", name="po3")
                    nc.tensor.matmul(
                        po3[:], wo_sb[:, 0, es], ctxT_sb[:, 0, qs],
                        start=True, stop=False,
                    )
                    nc.tensor.matmul(
                        po3[:], wo_sb[0:64, 1, es], ctxT_sb[0:64, 1, qs],
                        start=False, stop=True,
                    )
                    ot = out_pool.tile([128, 512], f32, tag="ot", name="ot")
                    busy["D"] += _cD(512)
                    nc.vector.tensor_copy(ot[:], po3[:])
                    nc.sync.dma_start(outT[es, qs], ot[:])

                slots = [(sq, h, kt)
                         for sq in range(NSQ)
                         for h in range(HL)
                         for kt in range(NKT)]
                pending = []

                def pop_one():
                    (s2, e2) = pending.pop(0)
                    emit_ctx(*s2, e2)

                vst = 0
                for j, slot in enumerate(slots):
                    # scores matmuls first in each slot so the exp engines'
                    # feed is never queue-delayed
                    expt = emit_scores_exp(*slot, j)
                    pending.append((slot, expt))
                    if vst < NST:
                        emit_vproj(vst)
                        vst += 1
                    # near the stream end the trail no longer buys slack --
                    # drain it so the final norm/out-proj/store chain starts
                    # as early as possible
                    trail_eff = TRAIL if j < len(slots) - 40 else 2
                    for _ in range(3):
                        if not pending:
                            break
                        need = (trail_eff + GAP if pending[0][0][2] == 0
                                else trail_eff)
                        if len(pending) > need:
                            pop_one()
                        else:
                            break
                    # drain one background / outproj op per slot (they share
                    # the sm psum ring so never interleave bg with outproj)
                    if bg_work and j >= bg_work[0][0]:
                        bg_work.pop(0)[1]()
                    elif outproj_work:
                        emit_outproj_tile(*outproj_work.pop(0))
                    # flex ops (chain tails, transposes, copies): 2-3 per slot
                    nfl = 3 if len(flex_work) > 10 else 2
                    for _ in range(nfl):
                        if not flex_work:
                            break
                        flex_work.pop(0)()
                while pending:
                    pop_one()
                    for _ in range(3):
                        if flex_work:
                            flex_work.pop(0)()
                    if outproj_work:
                        emit_outproj_tile(*outproj_work.pop(0))
                for _, op in bg_work:
                    op()
                while flex_work:
                    flex_work.pop(0)()
                while outproj_work:
                    emit_outproj_tile(*outproj_work.pop(0))

    nc.compile()
    return nc


def _get_nc():
    with _lock:
        if "nc" not in _compiled:
            _compiled["nc"] = _build()
        return _compiled["nc"]


def _prep_in_maps(query, key, value, prompt, Wq, bq, Wk, bk, Wv, bv, Wo, bo):
    f32 = np.float32
    qT = [np.ascontiguousarray(query[b].T).astype(BF16) for b in range(B)]
    kT = [np.ascontiguousarray(key[b].T).astype(BF16) for b in range(B)]
    vT = [np.ascontiguousarray(value[b].T).astype(BF16) for b in range(B)]
    ident = np.eye(128, dtype=BF16)
    in_maps = []
    for core in range(NCORES):
        b, g = core // NG, core % NG
        cs = slice(g * CL, (g + 1) * CL)
        kp = np.zeros((128, 2, PP), BF16)
        vpa = np.zeros((96, HL, D + 1), BF16)
        for h in range(HL):
            gh = g * HL + h
            kp[64 * (h % 2):64 * (h % 2) + 64, h // 2, :] = (
                prompt[b, 0, :, gh, :].T.astype(BF16))
            vpa[32 * h:32 * h + PP, h, 0:D] = (
                prompt[b, 1, :, gh, :].astype(BF16))
            vpa[32 * h:32 * h + PP, h, D] = 1.0
        in_maps.append({
            "xqT": qT[b], "xkT": kT[b], "xvT": vT[b],
            "wqT": np.ascontiguousarray(Wq[cs, :].T).astype(BF16),
            "wkT": np.ascontiguousarray(Wk[cs, :].T).astype(BF16),
            "wvT": np.ascontiguousarray(Wv[cs, :].T).astype(BF16),
            "woT": np.ascontiguousarray(Wo[:, cs].T).astype(BF16),
            "bq": np.ascontiguousarray(bq[cs]).astype(f32).reshape(CL, 1),
            "bk": np.ascontiguousarray(bk[cs]).astype(f32).reshape(CL, 1),
            "bv": np.ascontiguousarray(bv[cs]).astype(f32).reshape(1, CL),
            "kpT": kp, "vp3": vpa, "ident": ident,
        })
    return in_maps


def _combine(results, bo):
    out = np.empty((B, S, E), np.float32)
    for b in range(B):
        acc = results[b * NG]["outT"].astype(np.float32)
        for g in range(1, NG):
            acc = acc + results[b * NG + g]["outT"]
        out[b] = acc.T
    if bo is not None and np.any(bo):
        out += np.asarray(bo, np.float32)
    return out


def run(inputs, trace=False):
    """Returns (output, exec_time_ns or None)."""
    from concourse import bass_utils

    nc = _get_nc()
    in_maps = _prep_in_maps(**{k: np.asarray(v) for k, v in inputs.items()})
    bo = np.asarray(inputs["bo"])
    res = bass_utils.run_bass_kernel_spmd(
        nc, in_maps, core_ids=list(range(NCORES)), trace=trace,
    )
    return _combine(res.results, bo), res.exec_time_ns


def kernel(**inputs):
    out, _ = run(inputs)
    return out


# revision 37
# speedup vs baseline: 1.0175x; 1.0024x over previous
"""Multi-head attention (B=2,S=4096,E=768,H=12,D=64 + 16-token K/V prompt
prefix) on 8 Trainium2 NeuronCores.

Sharding: 2 batches x 4 head-groups (3 heads each). Each core computes QKV
projections for its 3 heads, full attention over its batch, and a partial
output projection (its 192 ctx channels); the host sums the 4 partials per
batch.

Per-core kernel layout (cost model: matmul = out-free-size rows; activation/
DVE/Pool = free-size * cycle):
  qT[c,s]   = Wq_g @ query^T            (lhsT=Wq_g^T chunks, rhs=queryT chunks)
  kT[c,s]   likewise; prompt K prefix kept separate (kpT)
  v[s,c]    natural orientation          (lhsT=valueT chunks, rhs=Wv_g^T)
  scoresT[k,q] = kT-tile^T @ qT          (lhsT=kT tile [64,128], rhs=qT [64,512])
  expT = Exp(scoresT / 8)                (ScalarE, or DVE/Pool via the
                                          (1+u+u^2/2)^8 chain, u=s/64)
  ctx[q, d+denom] += expT-slice^T @ v    (STATIONARY-SWAPPED: lhsT=expT
                                          [128kv,128q], rhs=v_aug [128kv,65]
                                          -> only 65 pumped rows per k-tile
                                          per q-chunk, half the PE cost of the
                                          [d,q] orientation)
  ctx_n[q,c] = ctx * recip(denom[q])     (per-partition scalar multiply)
  ctxT[c,q] = PE-transpose(ctx_n)        (via identity; rides scores psum ring)
  outT[e,q] partial = Wo_g^T-slices @ ctxT

Exp tiles are statically split across ScalarE / DVE / Pool so every engine's
modeled busy time lands near the PE's ~340us. Flexible evacuation ops
(normalize, psum->sbuf copies, out-proj evac) go through a greedy busy-time
balancer across DVE/Pool/ScalarE.
"""

import sys
import threading

import numpy as np

if "/opt/trn_rl_repo" not in sys.path:
    sys.path.insert(0, "/opt/trn_rl_repo")

import ml_dtypes

BF16 = ml_dtypes.bfloat16

B, S, E, H, D, PP = 2, 4096, 768, 12, 64, 16
NCORES = 8
NG = 4          # head-groups (tensor parallel)
HL = H // NG    # 3 local heads
CL = HL * D     # 192 local channels
SKV = PP + S    # 4112
NKT = S // 128  # 32 full k-tiles (prefix handled separately)
QT = 1024       # q tile width for scores/exp
NSQ = S // QT   # 4
NQC = QT // 128  # 8 q-chunks of 128 within a q tile
TRAIL = 12      # ctx matmuls trail scores by this many slots
GAP = 6         # extra trail for block-opening ctx (psc single-buffered)
NST = S // 128  # 32 v stiles

# --- engine split knobs ----------------------------------------------------
# (Pool/GPSIMD cannot access PSUM, so Pool-assigned exp tiles have their
#  first op (psum read) done by DVE; Pool runs the 6 SBUF-only chain ops.)
N_DVE_EXP = 0    # exp tiles handled end-to-end by the DVE 6-op chain
N_POOL_EXP = 0   # exp tiles: DVE op1 + Pool 6-op SBUF tail
# (Measured: every chain-offloaded tile ends up costing ~2.5us of scores-ring
#  stall -- the chain's first op queues behind DVE's in-order backlog and the
#  2-slot scores psum ring serializes on first-reader completion.  ScalarE-only
#  exp is the fastest measured configuration; the ctx stationary-swap keeps PE
#  ~80us below the baseline so ScalarE stays the sole pacer.)

# modeled per-op busy-time (ns) helpers for the greedy balancer
def _cS(f):       # ScalarE activation
    return (f + 222) * 0.8333

def _cD(f):       # DVE tensor op (psum involved)
    return f * 1.0417 + 125.0

def _cP(f, eff=0.6):  # Pool tensor op
    return f * 0.8333 / eff + 95.0

_lock = threading.Lock()
_compiled = {}


def _exp_engine_map():
    """Static slot -> engine ('S'/'D'/'P') assignment for the 384 exp tiles."""
    n_slots = NSQ * HL * NKT
    eng = ["S"] * n_slots
    # Pool tiles: long latency chain (13+ slots) -> keep away from stream ends
    for i in range(N_POOL_EXP):
        s = 40 + int(i * (n_slots - 108) / max(N_POOL_EXP, 1))
        eng[s] = "P"
    for i in range(N_DVE_EXP):
        s = 36 + int(i * (n_slots - 80) / max(N_DVE_EXP, 1))
        while eng[s] != "S":
            s += 1
        eng[s] = "D"
    return eng


def _build():
    import concourse.bass as bass  # noqa: F401
    import concourse.mybir as mybir
    import concourse.tile as tile
    from concourse import bacc

    f32 = mybir.dt.float32
    bf16 = mybir.dt.bfloat16
    EXP = mybir.ActivationFunctionType.Exp
    ALU = mybir.AluOpType

    nc = bacc.Bacc("TRN2", target_bir_lowering=False, debug=False)

    xqT = nc.dram_tensor("xqT", [E, S], bf16, kind="ExternalInput").ap()
    xkT = nc.dram_tensor("xkT", [E, S], bf16, kind="ExternalInput").ap()
    xvT = nc.dram_tensor("xvT", [E, S], bf16, kind="ExternalInput").ap()
    wqT = nc.dram_tensor("wqT", [E, CL], bf16, kind="ExternalInput").ap()
    wkT = nc.dram_tensor("wkT", [E, CL], bf16, kind="ExternalInput").ap()
    wvT = nc.dram_tensor("wvT", [E, CL], bf16, kind="ExternalInput").ap()
    woT = nc.dram_tensor("woT", [CL, E], bf16, kind="ExternalInput").ap()
    bq = nc.dram_tensor("bq", [CL, 1], f32, kind="ExternalInput").ap()
    bk = nc.dram_tensor("bk", [CL, 1], f32, kind="ExternalInput").ap()
    bv = nc.dram_tensor("bv", [1, CL], f32, kind="ExternalInput").ap()
    kpT = nc.dram_tensor("kpT", [128, 2, PP], bf16, kind="ExternalInput").ap()
    # vp3[:, h, :]: head h prefix V at rows 16h..16h+16 (zeros elsewhere),
    # col 64 = ones on those rows.  Zero-padding makes the prefix-ctx matmul
    # a plain full-contraction matmul (other heads' expp rows hit zeros).
    vp3 = nc.dram_tensor("vp3", [96, HL, D + 1], bf16,
                         kind="ExternalInput").ap()
    ident = nc.dram_tensor("ident", [128, 128], bf16, kind="ExternalInput").ap()
    outT = nc.dram_tensor("outT", [E, S], f32, kind="ExternalOutput").ap()

    busy = {"S": 0.0, "D": 0.0, "P": 0.0}
    exp_eng = _exp_engine_map()

    with tile.TileContext(nc) as tc:
        with tc.tile_pool(name="persist", bufs=1) as pers:
            # q-projection weights/bias first: they gate the very first
            # matmuls, so don't queue them behind the other ~1MB of DMAs
            wq_sb = pers.tile([128, 6, CL], bf16)
            nc.sync.dma_start(wq_sb[:], wqT.rearrange("(t p) c -> p t c", p=128))
            bq_sb = pers.tile([128, 2], f32)
            nc.sync.dma_start(bq_sb[:, 0:1], bq[0:128, :])
            nc.sync.dma_start(bq_sb[0:64, 1:2], bq[128:CL, :])

            wk_sb = pers.tile([128, 6, CL], bf16)
            wv_sb = pers.tile([128, 6, CL], bf16)
            wo_sb = pers.tile([128, 2, E], bf16)
            bk_sb = pers.tile([128, 2], f32)
            bvb_sb = pers.tile([128, CL], f32)
            kpT_sb = pers.tile([128, 2, PP], bf16)
            vp_sb = pers.tile([96, HL, D + 1], bf16)
            id_sb = pers.tile([128, 128], bf16)

            # activations (all bf16)
            qT_sb = pers.tile([128, 2, S], bf16)
            kT_sb = pers.tile([128, 2, S], bf16)   # no prefix; kpT separate
            v_sb = pers.tile([128, NST, HL, D + 1], bf16)
            ctxT_sb = pers.tile([128, 2, S], bf16)
            # prefix exp rows: head h at partitions 32h..32h+15
            # (gap rows zeroed once so the full-contraction prefix-ctx
            #  matmul contracts them harmlessly)
            expp_sb = pers.tile([96, S], bf16)

            nc.vector.memset(v_sb[:, :, :, D:D + 1], 1.0)
            nc.vector.memset(expp_sb[:], 0.0)

            # ---------------- Phase 1a: Q / K projections ----------------
            with (
                tc.tile_pool(name="ps_proj", bufs=2, space="PSUM") as pp,
                tc.tile_pool(name="xq_pool", bufs=4) as xq_pool,
            ):
                def proj_block(xin, wsb, bsb, dst, sq, skip_p1=False):
                    p0 = pp.tile([128, QT], f32, tag="p0", name="p0")
                    if not skip_p1:
                        p1 = pp.tile([64, QT], f32, tag="p1", name="p1")
                    for ech in range(6):
                        xt = xq_pool.tile([128, QT], bf16, tag="xt",
                                          name="xt")
                        nc.sync.dma_start(
                            xt[:],
                            xin[ech * 128:(ech + 1) * 128,
                                sq * QT:(sq + 1) * QT],
                        )
                        for n in range(QT // 512):
                            ns = slice(n * 512, (n + 1) * 512)
                            nc.tensor.matmul(
                                p0[:, ns], wsb[:, ech, 0:128], xt[:, ns],
                                start=(ech == 0), stop=(ech == 5),
                            )
                            if not skip_p1:
                                nc.tensor.matmul(
                                    p1[:, ns], wsb[:, ech, 128:CL],
                                    xt[:, ns],
                                    start=(ech == 0), stop=(ech == 5),
                                )
                    ds = slice(sq * QT, (sq + 1) * QT)
                    nc.vector.tensor_scalar_add(
                        dst[:, 0, ds], p0[:], bsb[:, 0:1])
                    if not skip_p1:
                        nc.vector.tensor_scalar_add(
                            dst[0:64, 1, ds], p1[:], bsb[0:64, 1:2])

                proj_block(xqT, wq_sb, bq_sb, qT_sb, 0)
                # now that the critical q DMAs are queued, stream in the
                # remaining weights behind them
                nc.sync.dma_start(
                    wk_sb[:], wkT.rearrange("(t p) c -> p t c", p=128))
                nc.sync.dma_start(bk_sb[:, 0:1], bk[0:128, :])
                nc.sync.dma_start(bk_sb[0:64, 1:2], bk[128:CL, :])
                nc.sync.dma_start(kpT_sb[:], kpT[:])
                nc.sync.dma_start(
                    wv_sb[:], wvT.rearrange("(t p) c -> p t c", p=128))
                nc.sync.dma_start(bvb_sb[:], bv.to_broadcast((128, CL)))
                nc.sync.dma_start(vp_sb[:], vp3[:])
                nc.sync.dma_start(id_sb[:], ident[:])
                nc.sync.dma_start(wo_sb[:, 0, :], woT[0:128, :])
                nc.sync.dma_start(wo_sb[0:64, 1, :], woT[128:CL, :])

                # prompt-prefix scores+exp (per head, like the baseline) --
                # exp rows land at expp partitions 16h..16h+16.  Starts
                # ScalarE (and its one-time exp table load) early.
                def emit_prefix(sq, psum_pool, tag):
                    # all 3 heads' prefix scores in one [96, QT] psum via
                    # quadrant tile positions -> a single ScalarE exp.  Gap
                    # rows hold exp(stale-psum) (finite); the prefix-ctx
                    # contracts them against vp3's zero rows.
                    psp = psum_pool.tile([96, QT], f32, tag=tag, name="psp")
                    # zero first: gap rows must exp() to something finite
                    nc.vector.memset(psp[:], 0.0)
                    for h in range(HL):
                        pr, po = h // 2, 64 * (h % 2)
                        for n in range(QT // 512):
                            ns = slice(n * 512, (n + 1) * 512)
                            qs = slice(sq * QT + n * 512,
                                       sq * QT + (n + 1) * 512)
                            nc.tensor.matmul(
                                psp[32 * h:32 * h + PP, ns],
                                kpT_sb[po:po + 64, pr, :],
                                qT_sb[po:po + 64, pr, qs],
                                start=True, stop=True,
                                tile_position=(po, 32 * h),
                            )
                    nc.scalar.activation(
                        expp_sb[:, sq * QT:(sq + 1) * QT], psp[:],
                        EXP, scale=float(D) ** -0.5,
                    )
                    busy["S"] += _cS(QT)

                emit_prefix(0, pp, "p0")

                for sq in range(NSQ):
                    proj_block(xkT, wk_sb, bk_sb, kT_sb, sq)

            # ---------- attention + V-proj + out-proj: one slot stream ----
            with (
                tc.tile_pool(name="ps_s", bufs=2, space="PSUM") as ps_s,
                tc.tile_pool(name="ps_c", bufs=1, space="PSUM") as ps_c,
                tc.tile_pool(name="ps_sm", bufs=2, space="PSUM") as ps_sm,
                tc.tile_pool(name="expt_pool", bufs=22) as expt_pool,
                tc.tile_pool(name="dve_scr", bufs=1) as dve_scr,
                tc.tile_pool(name="pool_scr", bufs=1) as pool_scr,
                tc.tile_pool(name="ctxn_pool", bufs=10) as ctxn_pool,
                tc.tile_pool(name="rc_pool", bufs=2) as rc_pool,
                tc.tile_pool(name="xv_pool", bufs=8) as xv_pool,
                tc.tile_pool(name="xq2_pool", bufs=6) as xq2_pool,
                tc.tile_pool(name="out_pool", bufs=4) as out_pool,
            ):
                def pick(cands):
                    """cands: {eng: cost_ns} -> engine with min projected busy."""
                    e = min(cands, key=lambda k: busy[k] + cands[k])
                    busy[e] += cands[e]
                    return e

                # Background q-projection for sq 1..3 (op-granular, drained
                # one op per stream slot using the time-multiplexed sm pool)
                def make_bg_qproj(sq):
                    ops = []
                    state = {}

                    def dma_op():
                        tiles = []
                        for ech in range(6):
                            xt2 = xq2_pool.tile([128, QT], bf16, tag="xt2",
                                                name="xt2")
                            nc.sync.dma_start(
                                xt2[:],
                                xqT[ech * 128:(ech + 1) * 128,
                                    sq * QT:(sq + 1) * QT],
                            )
                            tiles.append(xt2)
                        state["xt"] = tiles

                    ops.append(dma_op)

                    def mk_group(c, grp):
                        def op():
                            pt = ps_sm.tile([128, 512], f32, tag="sm",
                                            name="pq")
                            rows = 128 if grp == 0 else 64
                            wc = slice(0, 128) if grp == 0 else slice(128, CL)
                            for ech in range(6):
                                nc.tensor.matmul(
                                    pt[0:rows, :], wq_sb[:, ech, wc],
                                    state["xt"][ech][:, c * 512:(c + 1) * 512],
                                    start=(ech == 0), stop=(ech == 5),
                                )
                            qs = slice(sq * QT + c * 512,
                                       sq * QT + (c + 1) * 512)
                            busy["D"] += _cD(512)
                            if grp == 0:
                                dst, srcp, bias = (qT_sb[:, 0, qs], pt[:, :],
                                                   bq_sb[:, 0:1])
                            else:
                                dst, srcp, bias = (qT_sb[0:64, 1, qs],
                                                   pt[0:64, :],
                                                   bq_sb[0:64, 1:2])
                            nc.vector.tensor_scalar_add(dst, srcp, bias)
                        return op

                    for c in range(QT // 512):
                        for grp in range(2):
                            ops.append(mk_group(c, grp))
                    ops.append(lambda: emit_prefix(sq, ps_s, "pss"))
                    return ops

                bg_work = []
                for nb, sqb in ((32, 1), (70, 2), (150, 3)):
                    for op in make_bg_qproj(sqb):
                        bg_work.append((nb, op))

                # xv DMA loads, one sq-group of 6 chunks at a time
                xvts = {}

                def load_xv(sqx):
                    tiles = []
                    for ech in range(6):
                        xvt = xv_pool.tile([128, QT], bf16, tag="xvt",
                                           name="xvt")
                        nc.sync.dma_start(
                            xvt[:],
                            xvT[ech * 128:(ech + 1) * 128,
                                sqx * QT:(sqx + 1) * QT],
                        )
                        tiles.append(xvt)
                    xvts[sqx] = tiles

                def emit_vproj(st):
                    sqx, stl = st // (QT // 128), st % (QT // 128)
                    if st == 0:
                        load_xv(0)
                    if stl == 0 and sqx + 1 < NSQ:
                        load_xv(sqx + 1)
                    pv = ps_sm.tile([128, 512], f32, tag="sm", name="pv")
                    for ech in range(6):
                        nc.tensor.matmul(
                            pv[:, 0:CL],
                            xvts[sqx][ech][:, stl * 128:(stl + 1) * 128],
                            wv_sb[:, ech, :],
                            start=(ech == 0), stop=(ech == 5),
                        )
                    busy["D"] += _cD(CL)
                    nc.vector.tensor_add(
                        v_sb[:, st, :, 0:D],
                        pv[:, 0:CL].rearrange("p (h d) -> p h d", h=HL),
                        bvb_sb[:].rearrange("p (h d) -> p h d", h=HL),
                    )
                    if stl == (QT // 128) - 1:
                        del xvts[sqx]

                def emit_scores_exp(sq, h, kt, slot_idx):
                    pr, po = h // 2, 64 * (h % 2)
                    lhsT_k = kT_sb[po:po + 64, pr, kt * 128:(kt + 1) * 128]
                    pss = ps_s.tile([128, QT], f32, tag="pss", name="pss")
                    expt = expt_pool.tile([128, QT], bf16, tag="expt",
                                          name="expt")
                    for n in range(QT // 512):
                        ns = slice(n * 512, (n + 1) * 512)
                        qs = slice(sq * QT + n * 512, sq * QT + (n + 1) * 512)
                        nc.tensor.matmul(
                            pss[:, ns], lhsT_k, qT_sb[po:po + 64, pr, qs],
                            start=True, stop=True,
                        )
                    e = exp_eng[slot_idx]
                    if e == "S":
                        nc.scalar.activation(
                            expt[:], pss[:], EXP, scale=float(D) ** -0.5,
                        )
                        busy["S"] += _cS(QT)
                    elif e == "D":
                        # exp(s/8) ~ (1 + u + u^2/2)^8, u = s/64.
                        # op1 runs inline (it frees the scores psum slot);
                        # the 5 tail ops are spread via the flex queue so
                        # DVE's in-order queue stays shallow and never holds
                        # the scores ring hostage.
                        u = dve_scr.tile([128, QT], bf16, tag="du",
                                         name="du", bufs=2)
                        nc.vector.tensor_scalar(
                            u[:], pss[:], 1.0 / 64.0, None, ALU.mult)
                        a = dve_scr.tile([128, QT], bf16, tag="da",
                                         name="da", bufs=2)
                        t = dve_scr.tile([128, QT], bf16, tag="dt",
                                         name="dt", bufs=2)
                        s1 = dve_scr.tile([128, QT], bf16, tag="da",
                                          name="s1", bufs=2)
                        s2 = dve_scr.tile([128, QT], bf16, tag="dt",
                                          name="s2", bufs=2)
                        flex_work.append(lambda: nc.vector.scalar_tensor_tensor(
                            a[:], u[:], 0.5, u[:], ALU.mult, ALU.mult))
                        flex_work.append(lambda: nc.vector.scalar_tensor_tensor(
                            t[:], a[:], 1.0, u[:], ALU.add, ALU.add))
                        flex_work.append(lambda: nc.vector.tensor_mul(
                            s1[:], t[:], t[:]))
                        flex_work.append(lambda: nc.vector.tensor_mul(
                            s2[:], s1[:], s1[:]))
                        flex_work.append(lambda: nc.vector.tensor_mul(
                            expt[:], s2[:], s2[:]))
                        busy["D"] += _cD(QT) + 5 * (QT * 0.26 + 60.0)
                    else:
                        # Pool can't read PSUM: DVE does op1, Pool the rest
                        u = pool_scr.tile([128, QT], bf16, tag="pu",
                                          name="pu", bufs=3)
                        nc.vector.tensor_scalar(
                            u[:], pss[:], 1.0 / 64.0, None, ALU.mult)
                        busy["D"] += _cD(QT)
                        a = pool_scr.tile([128, QT], bf16, tag="pa",
                                          name="pa", bufs=2)
                        btl = pool_scr.tile([128, QT], bf16, tag="pb",
                                            name="btl", bufs=2)
                        t = pool_scr.tile([128, QT], bf16, tag="pa",
                                          name="t2", bufs=2)
                        s1 = pool_scr.tile([128, QT], bf16, tag="pb",
                                           name="s1", bufs=2)
                        s2 = pool_scr.tile([128, QT], bf16, tag="pa",
                                           name="s2", bufs=2)
                        flex_work.append(lambda: nc.gpsimd.tensor_mul(
                            a[:], u[:], u[:]))
                        flex_work.append(lambda: nc.gpsimd.tensor_scalar(
                            btl[:], a[:], 0.5, 1.0, ALU.mult, ALU.add))
                        flex_work.append(lambda: nc.gpsimd.tensor_add(
                            t[:], btl[:], u[:]))
                        flex_work.append(lambda: nc.gpsimd.tensor_mul(
                            s1[:], t[:], t[:]))
                        flex_work.append(lambda: nc.gpsimd.tensor_mul(
                            s2[:], s1[:], s1[:]))
                        flex_work.append(lambda: nc.gpsimd.tensor_mul(
                            expt[:], s2[:], s2[:]))
                        busy["P"] += _cP(QT) + 5 * _cP(QT, 0.42)
                    return expt

                flex_work = []   # norm/transpose/copy chains, drained per slot
                psc_tiles = {}

                def emit_ctx(sq, h, kt, expt):
                    key = (sq, h)
                    if kt == 0:
                        psc_tiles[key] = ps_c.tile([128, NQC, 128], f32,
                                                   tag="psc", name="psc")
                    psc = psc_tiles[key]
                    for i in range(NQC):
                        # start=True resets the WHOLE psum bank, so only the
                        # first region of each bank (4 regions/bank) may set
                        # it; the bank-wide zero covers the other regions.
                        nc.tensor.matmul(
                            psc[:, i, 0:D + 1],
                            expt[:, i * 128:(i + 1) * 128],
                            v_sb[:, kt, h, :],
                            start=(kt == 0 and i % 4 == 0),
                            stop=(kt == NKT - 1),
                            skip_group_check=True,
                        )
                    if kt == TRAIL - 1:
                        # prompt-prefix ctx contribution: full 48-row
                        # contraction; other heads' expp rows hit vp3 zeros
                        for i in range(NQC):
                            qs = slice(sq * QT + i * 128,
                                       sq * QT + (i + 1) * 128)
                            nc.tensor.matmul(
                                psc[:, i, 0:D + 1],
                                expp_sb[:, qs],
                                vp_sb[:, h, :],
                                start=False, stop=False,
                            )
                    if kt == NKT - 1:
                        queue_norm(sq, h, psc)
                        del psc_tiles[key]

                def queue_norm(sq, h, psc):
                    # recip + the 8 normalize multiplies run inline so psc
                    # frees quickly (it is single-buffered); the transpose +
                    # ctxT copies are queued -- they only gate out-proj.
                    pr, po = h // 2, 64 * (h % 2)
                    rc = rc_pool.tile([128, NQC, 1], f32, tag="rc", name="rc")
                    nc.vector.reciprocal(rc[:], psc[:, :, D:D + 1])
                    busy["D"] += _cD(NQC)
                    ctxns = []
                    for i in range(NQC):
                        ctxn = ctxn_pool.tile([128, D], bf16, tag="cn",
                                              name="ctxn")
                        busy["D"] += _cD(D)
                        nc.vector.tensor_scalar_mul(
                            ctxn[:], psc[:, i, 0:D], rc[:, i, 0:1])
                        ctxns.append(ctxn)

                    def mk_chunk(i):
                        def op():
                            ptr = ps_sm.tile([64, 128], bf16, tag="sm",
                                             name="ptr")
                            nc.tensor.transpose(ptr[:], ctxns[i][:], id_sb[:])
                            qs = slice(sq * QT + i * 128,
                                       sq * QT + (i + 1) * 128)
                            busy["D"] += _cD(128)
                            nc.vector.tensor_copy(
                                ctxT_sb[po:po + 64, pr, qs], ptr[:])
                        return op

                    for i in range(NQC):
                        flex_work.append(mk_chunk(i))
                    if h == HL - 1:
                        flex_work.append(lambda: emit_outproj(sq))

                outproj_work = []

                def emit_outproj(sq):
                    # queue the 12 out-projection tiles; drained 1/slot so
                    # they never lump up in front of scores matmuls
                    for et in range(6):
                        for n in range(QT // 512):
                            outproj_work.append((et, sq * 2 + n))

                def emit_outproj_tile(et, qn):
                    es = slice(et * 128, (et + 1) * 128)
                    qs = slice(qn * 512, (qn + 1) * 512)
                    po3 = ps_sm.tile([128, 512], f32, tag="sm", name="po3")
                    nc.tensor.matmul(
                        po3[:], wo_sb[:, 0, es], ctxT_sb[:, 0, qs],
                        start=True, stop=False,
                    )
                    nc.tensor.matmul(
                        po3[:], wo_sb[0:64, 1, es], ctxT_sb[0:64, 1, qs],
                        start=False, stop=True,
                    )
                    ot = out_pool.tile([128, 512], f32, tag="ot", name="ot")
                    busy["D"] += _cD(512)
                    nc.vector.tensor_copy(ot[:], po3[:])
                    nc.sync.dma_start(outT[es, qs], ot[:])

                slots = [(sq, h, kt)
                         for sq in range(NSQ)
                         for h in range(HL)
                         for kt in range(NKT)]
                pending = []

                def pop_one():
                    (s2, e2) = pending.pop(0)
                    emit_ctx(*s2, e2)

                vst = 0
                for j, slot in enumerate(slots):
                    # scores matmuls first in each slot so the exp engines'
                    # feed is never queue-delayed
                    expt = emit_scores_exp(*slot, j)
                    pending.append((slot, expt))
                    if vst < NST:
                        emit_vproj(vst)
                        vst += 1
                    # near the stream end the trail no longer buys slack --
                    # drain it so the final norm/out-proj/store chain starts
                    # as early as possible
                    trail_eff = TRAIL if j < len(slots) - 40 else 2
                    for _ in range(3):
                        if not pending:
                            break
                        need = (trail_eff + GAP if pending[0][0][2] == 0
                                else trail_eff)
                        if len(pending) > need:
                            pop_one()
                        else:
                            break
                    # drain background ops on alternate slots only: each is
                    # a ~1.3us PE lump, and spacing them keeps the 2-deep
                    # scores ring from starving ScalarE (bg and outproj share
                    # the sm psum ring so they never interleave)
                    if bg_work and j >= bg_work[0][0] and j % 2 == 0:
                        bg_work.pop(0)[1]()
                    elif outproj_work:
                        emit_outproj_tile(*outproj_work.pop(0))
                    # flex ops (chain tails, transposes, copies): 2-3 per slot
                    nfl = 3 if len(flex_work) > 10 else 2
                    for _ in range(nfl):
                        if not flex_work:
                            break
                        flex_work.pop(0)()
                while pending:
                    pop_one()
                    for _ in range(3):
                        if flex_work:
                            flex_work.pop(0)()
                    if outproj_work:
                        emit_outproj_tile(*outproj_work.pop(0))
                for _, op in bg_work:
                    op()
                while flex_work:
                    flex_work.pop(0)()
                while outproj_work:
                    emit_outproj_tile(*outproj_work.pop(0))

    nc.compile()
    return nc


def _get_nc():
    with _lock:
        if "nc" not in _compiled:
            _compiled["nc"] = _build()
        return _compiled["nc"]


def _prep_in_maps(query, key, value, prompt, Wq, bq, Wk, bk, Wv, bv, Wo, bo):
    f32 = np.float32
    qT = [np.ascontiguousarray(query[b].T).astype(BF16) for b in range(B)]
    kT = [np.ascontiguousarray(key[b].T).astype(BF16) for b in range(B)]
    vT = [np.ascontiguousarray(value[b].T).astype(BF16) for b in range(B)]
    ident = np.eye(128, dtype=BF16)
    in_maps = []
    for core in range(NCORES):
        b, g = core // NG, core % NG
        cs = slice(g * CL, (g + 1) * CL)
        kp = np.zeros((128, 2, PP), BF16)
        vpa = np.zeros((96, HL, D + 1), BF16)
        for h in range(HL):
            gh = g * HL + h
            kp[64 * (h % 2):64 * (h % 2) + 64, h // 2, :] = (
                prompt[b, 0, :, gh, :].T.astype(BF16))
            vpa[32 * h:32 * h + PP, h, 0:D] = (
                prompt[b, 1, :, gh, :].astype(BF16))
            vpa[32 * h:32 * h + PP, h, D] = 1.0
        in_maps.append({
            "xqT": qT[b], "xkT": kT[b], "xvT": vT[b],
            "wqT": np.ascontiguousarray(Wq[cs, :].T).astype(BF16),
            "wkT": np.ascontiguousarray(Wk[cs, :].T).astype(BF16),
            "wvT": np.ascontiguousarray(Wv[cs, :].T).astype(BF16),
            "woT": np.ascontiguousarray(Wo[:, cs].T).astype(BF16),
            "bq": np.ascontiguousarray(bq[cs]).astype(f32).reshape(CL, 1),
            "bk": np.ascontiguousarray(bk[cs]).astype(f32).reshape(CL, 1),
            "bv": np.ascontiguousarray(bv[cs]).astype(f32).reshape(1, CL),
            "kpT": kp, "vp3": vpa, "ident": ident,
        })
    return in_maps


def _combine(results, bo):
    out = np.empty((B, S, E), np.float32)
    for b in range(B):
        acc = results[b * NG]["outT"].astype(np.float32)
        for g in range(1, NG):
            acc = acc + results[b * NG + g]["outT"]
        out[b] = acc.T
    if bo is not None and np.any(bo):
        out += np.asarray(bo, np.float32)
    return out


def run(inputs, trace=False):
    """Returns (output, exec_time_ns or None)."""
    from concourse import bass_utils

    nc = _get_nc()
    in_maps = _prep_in_maps(**{k: np.asarray(v) for k, v in inputs.items()})
    bo = np.asarray(inputs["bo"])
    res = bass_utils.run_bass_kernel_spmd(
        nc, in_maps, core_ids=list(range(NCORES)), trace=trace,
    )
    return _combine(res.results, bo), res.exec_time_ns


def kernel(**inputs):
    out, _ = run(inputs)
    return out
